# revision 35
# baseline (speedup 1.0000x reference)
"""Multigrid F-cycle advection smoother on 8 TRN2 NeuronCores.

Strategy (zero-communication, redundant compute):
  - Column-shard: core k computes a [4096, 544] window of u:
    core 0 -> cols [0, 544), core k>0 -> cols [512k-32, 512k+512).
    Host keeps cols [0:512) from core 0, [32:544) from others.
    (Halo-32 windowing error ~5e-3 rel, validated vs f32 numpy ref.)
  - Host converts u to f16; DRAM I/O is f16 (halves DMA bytes, lets
    loads/stores use SP/ACT HWDGE queues in addition to GPSIMD SWDGE).
  - On-chip layout: partition p holds rows [32p, 32p+32); free = [row, col].
  - All intermediates f16 (f32 for the two coarsest restriction levels).
  - The error chain is NEGATED (N_j = -E_j) so the final correction is a
    plain add of a plain dup:  A'_j = -alpha_j*S_j - gamma_j (ACT affine),
      N_j = dup(N_{j+1}) + lam*(up0+left0 of dup) + A'_j
          = TT(up0+left0) ; stt(*lam + dup) ; TT(+A'_j)
    EC = coldup(N_1) is a pure copy; u += rowdup(EC) is a 2x TT.
  - TS = SQ1*T on ACT (quarter chunks); u += TS runs on GPSIMD.
  - Up-chain j=4..2 and j=1 run as row-half wavefronts: ACT prolongs one
    half while DVE chews the other; PSUM halo evacuations on DVE.
  - On non-final steps the second u += rowdup(EC) half rides the idle DMA
    engines as a GPSIMD SWDGE accumulate, freeing DVE for the next step.
  - Cross-partition halo rows via TensorE identity matmul.
"""
import numpy as np

import concourse.bass as bass
import concourse.bacc as bacc
import concourse.mybir as mybir
from concourse import tile
from concourse.bass_utils import run_bass_kernel_spmd
from concourse.masks import make_identity

F32 = mybir.dt.float32
F16 = mybir.dt.float16
ADD = mybir.AluOpType.add
MULT = mybir.AluOpType.mult
COPY_F = mybir.ActivationFunctionType.Copy
IDENT_F = mybir.ActivationFunctionType.Identity

NROW, WCOL, OWN, HALO = 4096, 544, 512, 32
DIAG = 1.2
LAM = float(np.float32(0.1 / DIAG))   # lam = SQ1 = 0.1/DIAG
SQ1 = LAM
ALPHA = {j: float(np.float32(-0.1 * 4.0 ** (-j) / DIAG)) for j in range(1, 6)}
LD = {j: (32 >> j, WCOL >> j) for j in range(1, 6)}  # per-partition (rl, cl)

_NC_CACHE = {}


def _v3(ap, c):
    return ap.rearrange("p (r c) -> p r c", c=c)


def _build(t_steps: int):
    nc = bacc.Bacc(None, target_bir_lowering=False)
    u_ext = nc.declare_dram_parameter("u", [NROW, WCOL], F16, isOutput=False)
    c_ext = nc.declare_dram_parameter("consts", [128, 8], F32, isOutput=False)
    out_ext = nc.declare_dram_parameter("out", [NROW, WCOL], F16, isOutput=True)

    u_dram = u_ext[:, :].rearrange("(p r) c -> p r c", p=128)
    out_dram = out_ext[:, :].rearrange("(p r) c -> p r c", p=128)

    DVE, GPS, ACT, PE = nc.vector, nc.gpsimd, nc.scalar, nc.tensor

    with tile.TileContext(nc) as tc:
        with tc.tile_pool(name="sb", bufs=1) as pool, \
             tc.tile_pool(name="ps", bufs=1, space="PSUM") as ppool:
            ua = pool.tile([128, 33 * WCOL], F16, name="ua")   # row 0 = up halo
            T = pool.tile([128, 32 * WCOL], F16, name="t")
            TS = pool.tile([128, 32 * WCOL], F16, name="ts")
            RP = pool.tile([128, 16 * WCOL], F16, name="rp")   # also EC at step end
            S = {1: pool.tile([128, 16 * (WCOL >> 1)], F16, name="s1")}
            RA = {}
            for j in range(2, 6):
                rl, cl = LD[j]
                dt = F32 if j >= 4 else F16
                RA[j] = pool.tile([128, max(rl * 2 * cl, 8)], dt, name=f"ra{j}")
                S[j] = pool.tile([128, max(rl * cl, 8)], dt, name=f"s{j}")
            A = {j: pool.tile([128, max(LD[j][0] * LD[j][1], 8)], F16,
                              name=f"a{j}") for j in range(1, 5)}
            # N_j = -E_j tiles; scratch U_j for the shift sums
            N = {j: pool.tile([128, max(LD[j][0] * LD[j][1], 8)], F16, name=f"n{j}")
                 for j in range(1, 6)}
            U = {j: pool.tile([128, max(LD[j][0] * LD[j][1], 8)], F16, name=f"u{j}")
                 for j in range(1, 5)}
            P = {j: pool.tile([128, (1 + LD[j][0]) * (LD[j][1] + 2)], F16,
                              name=f"p{j}") for j in range(1, 5)}
            zsh = pool.tile([128, 130], F16, name="zsh")
            consts = pool.tile([128, 8], F32, name="k")
            psum = ppool.tile([128, 1024], F32, name="hps")

            nc.sync.dma_start(consts[:, :], c_ext[:, :])
            DVE.memset(zsh[:, :], 0.0)
            make_identity(nc, zsh[:, 1:129], nomemset=True)
            uav = _v3(ua[:, :], WCOL)
            # chunked load: 4 row-groups of 8 rows, one DMA queue each
            ld_eng = [GPS, nc.sync, ACT, nc.sync]
            for ch in range(4):
                ld_eng[ch].dma_start(uav[:, 1 + 8 * ch:9 + 8 * ch, :],
                                     u_dram[:, 8 * ch:8 + 8 * ch, :])

            g_ap = {j: consts[:, j - 1:j] for j in range(1, 6)}
            # zero left-pad column of every P tile (once; never rewritten)
            for j in range(1, 5):
                rl, cl = LD[j]
                Pv = _v3(P[j][:, :], cl + 2)
                zc = Pv[:, :, 0:1].rearrange("p r one -> p (r one)")
                DVE.memset(zc, 0.0)

            T3 = _v3(T[:, :], WCOL)
            TS3 = _v3(TS[:, :], WCOL)
            Tp = T[:, :].rearrange("p (rp two c) -> p rp two c", two=2, c=WCOL)
            RP3 = _v3(RP[:, :], WCOL)
            RPc = RP[:, :].rearrange("p (r cp two) -> p r cp two", r=16, two=2)
            S13 = _v3(S[1][:, :], WCOL >> 1)
            ur = uav[:, 1:33, :].rearrange("p (rp two) c -> p rp two c", two=2)
            N13 = _v3(N[1][:, :], WCOL >> 1)
            EC3 = RP3  # EC aliases RP ([p, 16, WCOL])

            def t_rows(r0, r1):
                """T = u_up + u_left for stripe rows [r0, r1)."""
                DVE.tensor_tensor(T3[:, r0:r1, 1:WCOL], uav[:, r0:r1, 1:WCOL],
                                  uav[:, r0 + 1:r1 + 1, 0:WCOL - 1], ADD)
                DVE.tensor_tensor(T3[:, r0:r1, 0:1], uav[:, r0:r1, 0:1],
                                  uav[:, r0 + 1:r1 + 1, 0:1], ADD)

            def u_halo():
                PE.matmul(psum[:, 0:512], zsh[:, 0:128], uav[:, 32, 0:512])
                PE.matmul(psum[:, 512:WCOL], zsh[:, 0:128], uav[:, 32, 512:WCOL])
                ACT.activation(uav[:, 0, :], psum[:, 0:WCOL], COPY_F)

            def p0_copy():
                # global-top replicate BC for partition 0 (DVE: tiny, and
                # keeps it out of ACT's TS-busy queue)
                DVE.tensor_copy(uav[0:1, 0, :], uav[0:1, 1, :])

            for _step in range(t_steps):
                first = _step == 0
                if first:
                    # overlap the load: rows 1.. of each chunk need no halo
                    t_rows(1, 8)
                    for ch in range(1, 4):
                        t_rows(8 * ch, 8 * ch + 8)
                    u_halo()           # waits only on last-loaded chunk
                    p0_copy()
                    t_rows(0, 1)
                else:
                    u_halo()
                    t_rows(1, 16)
                    t_rows(16, 32)
                    p0_copy()
                    t_rows(0, 1)

                # ---- TS = SQ1 * T (ACT); u += TS (GPS) ----
                # rows 0:28 now; rows 28:32 deferred into ACT's idle window
                # during the j=1 prolongation (keeps ACT free when S5 lands)
                def ts_g(r0, r1):
                    ACT.activation(TS3[:, r0:r1, :], T3[:, r0:r1, :], COPY_F,
                                   scale=SQ1)
                    u_flat = ua[:, (1 + r0) * WCOL:(1 + r1) * WCOL]
                    GPS.tensor_tensor(u_flat, u_flat,
                                      TS[:, r0 * WCOL:r1 * WCOL], ADD)

                for q in range(4):
                    ts_g(8 * q, 8 * q + 8)

                # ---- restriction: RP = rowpair(T); S1 = colpair(RP) ----
                DVE.tensor_tensor(RP3[:, :, :], Tp[:, :, 0, :], Tp[:, :, 1, :],
                                  ADD)
                DVE.tensor_tensor(S13[:, :, :], RPc[:, :, :, 0],
                                  RPc[:, :, :, 1], ADD)
                prev = S[1]
                for j in range(2, 6):
                    rl, cl = LD[j]
                    pv = prev[:, 0:rl * 4 * cl].rearrange(
                        "p (rp two c) -> p rp two c", two=2, c=2 * cl)
                    ra3 = _v3(RA[j][:, 0:rl * 2 * cl], 2 * cl)
                    DVE.tensor_tensor(ra3[:, :, :], pv[:, :, 0, :],
                                      pv[:, :, 1, :], ADD)
                    rac = RA[j][:, 0:rl * 2 * cl].rearrange(
                        "p (r cp two) -> p r cp two", r=rl, two=2)
                    s3 = _v3(S[j][:, 0:rl * cl], cl)
                    DVE.tensor_tensor(s3[:, :, :], rac[:, :, :, 0],
                                      rac[:, :, :, 1], ADD)
                    prev = S[j]

                # ---- A'_j = -(alpha_j*S_j + gamma_j) (ACT affine) ----
                # N5 = A'_5 directly; emitted in need order (coarse first)
                n5 = LD[5][1]
                ACT.activation(N[5][:, 0:n5], S[5][:, 0:n5], IDENT_F,
                               bias=g_ap[5], scale=-ALPHA[5])
                for j in range(4, 0, -1):
                    rl, cl = LD[j]
                    ACT.activation(A[j][:, 0:rl * cl], S[j][:, 0:rl * cl],
                                   IDENT_F, bias=g_ap[j], scale=-ALPHA[j])

                # ---- up chain j=4..2: N_j = dup + lam*(up0+left0) + A' ----
                # row-half wavefront: ACT prolongs half rh while DVE chews
                # the previous half, so ACT latency hides behind DVE work.
                for j in range(4, 1, -1):
                    rl, cl = LD[j]
                    r2, c2 = LD[j + 1]
                    rlh, r2h = rl // 2, max(r2 // 2, 1)
                    Pv = _v3(P[j][:, :], cl + 2)
                    Nin3 = _v3(N[j + 1][:, 0:r2 * c2], c2)
                    Uj3 = _v3(U[j][:, 0:rl * cl], cl)
                    Nj3 = _v3(N[j][:, 0:rl * cl], cl)
                    Aj3 = _v3(A[j][:, 0:rl * cl], cl)
                    for rh in range(2):
                        h0, h1 = rlh * rh, rlh * (rh + 1)
                        # prolong P rows [h0+1, h1+1) <- dup(N_{j+1} rows)
                        if rlh == 1:
                            # j=4: each half is one row, dup of N5 row 0
                            Ed1 = Nin3[:, 0, :].unsqueeze(2).broadcast_to(
                                [128, c2, 2])
                            Vr1 = Pv[:, h0 + 1, 1:cl + 1].rearrange(
                                "p (c2 b) -> p c2 b", b=2)
                            ACT.activation(Vr1, Ed1, COPY_F)
                        else:
                            Vr = Pv[:, h0 + 1:h1 + 1, 1:cl + 1].rearrange(
                                "p (rr a) c -> p rr a c", a=2).rearrange(
                                "p rr a (c2 b) -> p rr a c2 b", b=2)
                            Edup = Nin3[:, r2h * rh:r2h * (rh + 1), :].\
                                unsqueeze(3).broadcast_to([128, r2h, c2, 2])
                            ACT.activation(Vr[:, :, 0, :, :], Edup, COPY_F)
                            ACT.activation(Vr[:, :, 1, :, :], Edup, COPY_F)
                        if rh == 0:
                            # halo row via PE; evacuation on DVE
                            Elast = Nin3[:, r2 - 1, :].unsqueeze(2).\
                                broadcast_to([128, c2, 2])
                            PE.matmul(psum[:, 0:cl], zsh[:, 0:128], Elast)
                            DVE.tensor_copy(Pv[:, 0, 1:cl + 1], psum[:, 0:cl])
                        # chain (DVE): U = up0+left0; N = U*lam+dup; N += A'
                        DVE.tensor_tensor(Uj3[:, h0:h1, :],
                                          Pv[:, h0:h1, 1:cl + 1],
                                          Pv[:, h0 + 1:h1 + 1, 0:cl], ADD)
                        DVE.scalar_tensor_tensor(
                            U[j][:, h0 * cl:h1 * cl], U[j][:, h0 * cl:h1 * cl],
                            LAM, Pv[:, h0 + 1:h1 + 1, 1:cl + 1], MULT, ADD)
                        DVE.tensor_tensor(Nj3[:, h0:h1, :], Uj3[:, h0:h1, :],
                                          Aj3[:, h0:h1, :], ADD)

                # ---- j=1 + EC + u-add: row-half wavefront for the tail ----
                last = _step == t_steps - 1
                rl, cl = LD[1]
                r2, c2 = LD[2]
                Pv = _v3(P[1][:, :], cl + 2)
                N23 = _v3(N[2][:, 0:r2 * c2], c2)
                U13 = _v3(U[1][:, :], cl)
                A13 = _v3(A[1][:, :], cl)
                for rh in range(2):
                    h0, h1 = 8 * rh, 8 * rh + 8
                    # prolong P1 rows [h0+1, h1+1) <- dup(N2 rows [4rh,4rh+4))
                    # col-split: first ACT op covers the left half only
                    Vr = Pv[:, h0 + 1:h1 + 1, 1:cl + 1].rearrange(
                        "p (r2 a) c -> p r2 a c", a=2).rearrange(
                        "p r2 a (c2 b) -> p r2 a c2 b", b=2)
                    c2h = c2
                    for ch in range(1):
                        Edup = N23[:, 4 * rh:4 * rh + 4,
                                   c2h * ch:c2h * (ch + 1)].unsqueeze(3).\
                            broadcast_to([128, 4, c2h, 2])
                        Vc = Vr[:, :, :, c2h * ch:c2h * (ch + 1), :]
                        ACT.activation(Vc[:, :, 0, :, :], Edup, COPY_F)
                        ACT.activation(Vc[:, :, 1, :, :], Edup, COPY_F)
                    if rh == 0:
                        Elast = N23[:, r2 - 1, :].unsqueeze(2).broadcast_to(
                            [128, c2, 2])
                        PE.matmul(psum[:, 0:cl], zsh[:, 0:128], Elast)
                        DVE.tensor_copy(Pv[:, 0, 1:cl + 1], psum[:, 0:cl])
                    # DVE chain for this half
                    DVE.tensor_tensor(U13[:, h0:h1, :],
                                      Pv[:, h0:h1, 1:cl + 1],
                                      Pv[:, h0 + 1:h1 + 1, 0:cl], ADD)
                    DVE.scalar_tensor_tensor(
                        U[1][:, h0 * cl:h1 * cl], U[1][:, h0 * cl:h1 * cl],
                        LAM, Pv[:, h0 + 1:h1 + 1, 1:cl + 1], MULT, ADD)
                    DVE.tensor_tensor(N13[:, h0:h1, :], U13[:, h0:h1, :],
                                      A13[:, h0:h1, :], ADD)
                    # EC = coldup(N1 half) (ACT copy); u += rowdup(EC) (DVE)
                    E1dup = N13[:, h0:h1, :].unsqueeze(3).broadcast_to(
                        [128, 8, WCOL >> 1, 2])
                    ECr = EC3[:, h0:h1, :].rearrange(
                        "p r (cp two) -> p r cp two", two=2)
                    ACT.activation(ECr, E1dup, COPY_F)
                    if not last:
                        ECb = EC3[:, h0:h1, :].unsqueeze(2).broadcast_to(
                            [128, 8, 2, WCOL])
                        if rh == 0:
                            DVE.tensor_tensor(ur[:, h0:h1, :, :],
                                              ur[:, h0:h1, :, :], ECb, ADD)
                        else:
                            # ride the idle DMA engines: u += rowdup(EC)
                            # via SWDGE accumulate; DVE races into next step
                            GPS.dma_start(ur[:, h0:h1, 0, :],
                                          EC3[:, h0:h1, :], accum_op=ADD)
                            GPS.dma_start(ur[:, h0:h1, 1, :],
                                          EC3[:, h0:h1, :], accum_op=ADD)
                    else:
                        # 4-row final adds so the store stream starts early
                        st_eng = (GPS, nc.sync, ACT, nc.sync)
                        for sub in range(4):
                            q0 = h0 + 2 * sub
                            ECb = EC3[:, q0:q0 + 2, :].unsqueeze(2).\
                                broadcast_to([128, 2, 2, WCOL])
                            DVE.tensor_tensor(ur[:, q0:q0 + 2, :, :],
                                              ur[:, q0:q0 + 2, :, :], ECb, ADD)
                            r0 = 16 * rh + 4 * sub
                            st_eng[sub].dma_start(
                                out_dram[:, r0:r0 + 4, :],
                                uav[:, 1 + r0:5 + r0, :])

    nc.finalize()
    return nc


def _consts_np(bs: float, br: float) -> np.ndarray:
    # bias for A'_j = -(alpha_j*S_j + gamma_j): store -gamma_j
    c = np.zeros(8, np.float32)
    c[0] = np.float32(-(bs + br) / DIAG)
    for j in range(2, 6):
        c[j - 1] = np.float32(-j * br / DIAG)
    return np.tile(c[None, :], (128, 1))


def kernel(u, b_smooth, b_res, t):
    t = int(np.asarray(t))
    u = np.ascontiguousarray(np.asarray(u, np.float32))
    bs = float(np.asarray(b_smooth).reshape(-1)[0])
    br = float(np.asarray(b_res).reshape(-1)[0])
    u2 = u[0, 0]

    if t not in _NC_CACHE:
        _NC_CACHE[t] = _build(t)
    nc = _NC_CACHE[t]

    consts = _consts_np(bs, br)
    u16 = u2.astype(np.float16)
    in_maps = []
    for k in range(8):
        w = u16[:, 0:WCOL] if k == 0 else u16[:, OWN * k - HALO: OWN * k + OWN]
        in_maps.append({"u": np.ascontiguousarray(w), "consts": consts})

    res = run_bass_kernel_spmd(nc, in_maps, list(range(8)))
    outs = []
    for k in range(8):
        o = res.results[k]["out"]
        outs.append(o[:, 0:OWN] if k == 0 else o[:, HALO:WCOL])
    full = np.concatenate(outs, axis=1)
    return full[None, None].astype(np.float32)


# revision 37
# speedup vs baseline: 1.0187x; 1.0187x over previous
"""Multigrid F-cycle advection smoother on 8 TRN2 NeuronCores.

Strategy (zero-communication, redundant compute):
  - Column-shard: core k computes a [4096, 544] window of u:
    core 0 -> cols [0, 544), core k>0 -> cols [512k-32, 512k+512).
    Host keeps cols [0:512) from core 0, [32:544) from others.
    (Halo-32 windowing error ~5e-3 rel, validated vs f32 numpy ref.)
  - Host converts u to f16; DRAM I/O is f16 (halves DMA bytes, lets
    loads/stores use SP/ACT HWDGE queues in addition to GPSIMD SWDGE).
  - On-chip layout: partition p holds rows [32p, 32p+32); free = [row, col].
  - All intermediates f16 (f32 for the two coarsest restriction levels).
  - The error chain is NEGATED (N_j = -E_j) so the final correction is a
    plain add of a plain dup:  A'_j = -alpha_j*S_j - gamma_j (ACT affine),
      N_j = dup(N_{j+1}) + lam*(up0+left0 of dup) + A'_j
          = TT(up0+left0) ; stt(*lam + dup) ; TT(+A'_j)
    EC = coldup(N_1) is a pure copy; u += rowdup(EC) is a 2x TT.
  - TS = SQ1*T on ACT (quarter chunks); u += TS runs on GPSIMD.
  - Up-chain j=4..2 and j=1 run as row-half wavefronts: ACT prolongs one
    half while DVE chews the other; PSUM halo evacuations on DVE.
  - On non-final steps the second u += rowdup(EC) half rides the idle DMA
    engines as a GPSIMD SWDGE accumulate, freeing DVE for the next step.
  - Cross-partition halo rows via TensorE identity matmul.
"""
import numpy as np

import concourse.bass as bass
import concourse.bacc as bacc
import concourse.mybir as mybir
from concourse import tile
from concourse.bass_utils import run_bass_kernel_spmd
from concourse.masks import make_identity

F32 = mybir.dt.float32
F16 = mybir.dt.float16
ADD = mybir.AluOpType.add
MULT = mybir.AluOpType.mult
COPY_F = mybir.ActivationFunctionType.Copy
IDENT_F = mybir.ActivationFunctionType.Identity

NROW, WCOL, OWN, HALO = 4096, 544, 512, 32
DIAG = 1.2
LAM = float(np.float32(0.1 / DIAG))   # lam = SQ1 = 0.1/DIAG
SQ1 = LAM
ALPHA = {j: float(np.float32(-0.1 * 4.0 ** (-j) / DIAG)) for j in range(1, 6)}
LD = {j: (32 >> j, WCOL >> j) for j in range(1, 6)}  # per-partition (rl, cl)

_NC_CACHE = {}


def _v3(ap, c):
    return ap.rearrange("p (r c) -> p r c", c=c)


def _build(t_steps: int):
    nc = bacc.Bacc(None, target_bir_lowering=False)
    u_ext = nc.declare_dram_parameter("u", [NROW, WCOL], F16, isOutput=False)
    c_ext = nc.declare_dram_parameter("consts", [128, 8], F32, isOutput=False)
    out_ext = nc.declare_dram_parameter("out", [NROW, WCOL], F16, isOutput=True)

    u_dram = u_ext[:, :].rearrange("(p r) c -> p r c", p=128)
    out_dram = out_ext[:, :].rearrange("(p r) c -> p r c", p=128)

    DVE, GPS, ACT, PE = nc.vector, nc.gpsimd, nc.scalar, nc.tensor

    with tile.TileContext(nc) as tc:
        with tc.tile_pool(name="sb", bufs=1) as pool, \
             tc.tile_pool(name="ps", bufs=1, space="PSUM") as ppool:
            ua = pool.tile([128, 33 * WCOL], F16, name="ua")   # row 0 = up halo
            T = pool.tile([128, 32 * WCOL], F16, name="t")
            TS = pool.tile([128, 32 * WCOL], F16, name="ts")
            RP = pool.tile([128, 16 * WCOL], F16, name="rp")   # also EC at step end
            S = {1: pool.tile([128, 16 * (WCOL >> 1)], F16, name="s1")}
            RA = {}
            for j in range(2, 6):
                rl, cl = LD[j]
                dt = F32 if j >= 4 else F16
                RA[j] = pool.tile([128, max(rl * 2 * cl, 8)], dt, name=f"ra{j}")
                S[j] = pool.tile([128, max(rl * cl, 8)], dt, name=f"s{j}")
            A = {j: pool.tile([128, max(LD[j][0] * LD[j][1], 8)], F16,
                              name=f"a{j}") for j in range(1, 5)}
            # N_j = -E_j tiles; scratch U_j for the shift sums
            N = {j: pool.tile([128, max(LD[j][0] * LD[j][1], 8)], F16, name=f"n{j}")
                 for j in range(1, 6)}
            U = {j: pool.tile([128, max(LD[j][0] * LD[j][1], 8)], F16, name=f"u{j}")
                 for j in range(1, 5)}
            P = {j: pool.tile([128, (1 + LD[j][0]) * (LD[j][1] + 2)], F16,
                              name=f"p{j}") for j in range(1, 5)}
            zsh = pool.tile([128, 130], F16, name="zsh")
            consts = pool.tile([128, 8], F32, name="k")
            psum = ppool.tile([128, 1024], F32, name="hps")

            nc.sync.dma_start(consts[:, :], c_ext[:, :])
            DVE.memset(zsh[:, :], 0.0)
            make_identity(nc, zsh[:, 1:129], nomemset=True)
            uav = _v3(ua[:, :], WCOL)
            # chunked load: 4 row-groups of 8 rows, one DMA queue each
            ld_eng = [GPS, nc.sync, ACT, nc.sync]
            for ch in range(4):
                ld_eng[ch].dma_start(uav[:, 1 + 8 * ch:9 + 8 * ch, :],
                                     u_dram[:, 8 * ch:8 + 8 * ch, :])

            g_ap = {j: consts[:, j - 1:j] for j in range(1, 6)}
            # zero left-pad column of every P tile (once; never rewritten)
            for j in range(1, 5):
                rl, cl = LD[j]
                Pv = _v3(P[j][:, :], cl + 2)
                zc = Pv[:, :, 0:1].rearrange("p r one -> p (r one)")
                DVE.memset(zc, 0.0)

            T3 = _v3(T[:, :], WCOL)
            TS3 = _v3(TS[:, :], WCOL)
            Tp = T[:, :].rearrange("p (rp two c) -> p rp two c", two=2, c=WCOL)
            RP3 = _v3(RP[:, :], WCOL)
            RPc = RP[:, :].rearrange("p (r cp two) -> p r cp two", r=16, two=2)
            S13 = _v3(S[1][:, :], WCOL >> 1)
            ur = uav[:, 1:33, :].rearrange("p (rp two) c -> p rp two c", two=2)
            N13 = _v3(N[1][:, :], WCOL >> 1)
            EC3 = RP3  # EC aliases RP ([p, 16, WCOL])

            def t_rows(r0, r1):
                """T = u_up + u_left for stripe rows [r0, r1)."""
                DVE.tensor_tensor(T3[:, r0:r1, 1:WCOL], uav[:, r0:r1, 1:WCOL],
                                  uav[:, r0 + 1:r1 + 1, 0:WCOL - 1], ADD)
                DVE.tensor_tensor(T3[:, r0:r1, 0:1], uav[:, r0:r1, 0:1],
                                  uav[:, r0 + 1:r1 + 1, 0:1], ADD)

            def u_halo():
                PE.matmul(psum[:, 0:512], zsh[:, 0:128], uav[:, 32, 0:512])
                PE.matmul(psum[:, 512:WCOL], zsh[:, 0:128], uav[:, 32, 512:WCOL])
                ACT.activation(uav[:, 0, :], psum[:, 0:WCOL], COPY_F)

            def p0_copy():
                # global-top replicate BC for partition 0 (DVE: tiny, and
                # keeps it out of ACT's TS-busy queue)
                DVE.tensor_copy(uav[0:1, 0, :], uav[0:1, 1, :])

            for _step in range(t_steps):
                first = _step == 0
                if first:
                    # overlap the load: rows 1.. of each chunk need no halo
                    t_rows(1, 8)
                    for ch in range(1, 4):
                        t_rows(8 * ch, 8 * ch + 8)
                    u_halo()           # waits only on last-loaded chunk
                    p0_copy()
                    t_rows(0, 1)
                else:
                    u_halo()
                    t_rows(1, 16)
                    t_rows(16, 24)
                    t_rows(24, 32)
                    p0_copy()
                    t_rows(0, 1)

                # ---- TS = SQ1 * T (ACT); u += TS (GPS) ----
                # rows 0:28 now; rows 28:32 deferred into ACT's idle window
                # during the j=1 prolongation (keeps ACT free when S5 lands)
                def ts_g(r0, r1):
                    ACT.activation(TS3[:, r0:r1, :], T3[:, r0:r1, :], COPY_F,
                                   scale=SQ1)
                    u_flat = ua[:, (1 + r0) * WCOL:(1 + r1) * WCOL]
                    GPS.tensor_tensor(u_flat, u_flat,
                                      TS[:, r0 * WCOL:r1 * WCOL], ADD)

                for q in range(4):
                    ts_g(8 * q, 8 * q + 8)

                # ---- restriction: RP = rowpair(T); S1 = colpair(RP) ----
                DVE.tensor_tensor(RP3[:, :, :], Tp[:, :, 0, :], Tp[:, :, 1, :],
                                  ADD)
                DVE.tensor_tensor(S13[:, :, :], RPc[:, :, :, 0],
                                  RPc[:, :, :, 1], ADD)
                prev = S[1]
                for j in range(2, 6):
                    rl, cl = LD[j]
                    pv = prev[:, 0:rl * 4 * cl].rearrange(
                        "p (rp two c) -> p rp two c", two=2, c=2 * cl)
                    ra3 = _v3(RA[j][:, 0:rl * 2 * cl], 2 * cl)
                    DVE.tensor_tensor(ra3[:, :, :], pv[:, :, 0, :],
                                      pv[:, :, 1, :], ADD)
                    rac = RA[j][:, 0:rl * 2 * cl].rearrange(
                        "p (r cp two) -> p r cp two", r=rl, two=2)
                    s3 = _v3(S[j][:, 0:rl * cl], cl)
                    DVE.tensor_tensor(s3[:, :, :], rac[:, :, :, 0],
                                      rac[:, :, :, 1], ADD)
                    prev = S[j]

                # ---- A'_j = -(alpha_j*S_j + gamma_j) (ACT affine) ----
                # N5 = A'_5 directly; emitted in need order (coarse first)
                n5 = LD[5][1]
                ACT.activation(N[5][:, 0:n5], S[5][:, 0:n5], IDENT_F,
                               bias=g_ap[5], scale=-ALPHA[5])
                for j in range(4, 0, -1):
                    rl, cl = LD[j]
                    ACT.activation(A[j][:, 0:rl * cl], S[j][:, 0:rl * cl],
                                   IDENT_F, bias=g_ap[j], scale=-ALPHA[j])

                # ---- up chain j=4..2: N_j = dup + lam*(up0+left0) + A' ----
                # row-half wavefront: ACT prolongs half rh while DVE chews
                # the previous half, so ACT latency hides behind DVE work.
                for j in range(4, 1, -1):
                    rl, cl = LD[j]
                    r2, c2 = LD[j + 1]
                    rlh, r2h = rl // 2, max(r2 // 2, 1)
                    Pv = _v3(P[j][:, :], cl + 2)
                    Nin3 = _v3(N[j + 1][:, 0:r2 * c2], c2)
                    Uj3 = _v3(U[j][:, 0:rl * cl], cl)
                    Nj3 = _v3(N[j][:, 0:rl * cl], cl)
                    Aj3 = _v3(A[j][:, 0:rl * cl], cl)
                    for rh in range(2):
                        h0, h1 = rlh * rh, rlh * (rh + 1)
                        # prolong P rows [h0+1, h1+1) <- dup(N_{j+1} rows)
                        if rlh == 1:
                            # j=4: each half is one row, dup of N5 row 0
                            Ed1 = Nin3[:, 0, :].unsqueeze(2).broadcast_to(
                                [128, c2, 2])
                            Vr1 = Pv[:, h0 + 1, 1:cl + 1].rearrange(
                                "p (c2 b) -> p c2 b", b=2)
                            ACT.activation(Vr1, Ed1, COPY_F)
                        else:
                            Vr = Pv[:, h0 + 1:h1 + 1, 1:cl + 1].rearrange(
                                "p (rr a) c -> p rr a c", a=2).rearrange(
                                "p rr a (c2 b) -> p rr a c2 b", b=2)
                            Edup = Nin3[:, r2h * rh:r2h * (rh + 1), :].\
                                unsqueeze(3).broadcast_to([128, r2h, c2, 2])
                            ACT.activation(Vr[:, :, 0, :, :], Edup, COPY_F)
                            ACT.activation(Vr[:, :, 1, :, :], Edup, COPY_F)
                        if rh == 0:
                            # halo row via PE; evacuation on DVE
                            Elast = Nin3[:, r2 - 1, :].unsqueeze(2).\
                                broadcast_to([128, c2, 2])
                            PE.matmul(psum[:, 0:cl], zsh[:, 0:128], Elast)
                            DVE.tensor_copy(Pv[:, 0, 1:cl + 1], psum[:, 0:cl])
                        # chain (DVE): U = up0+left0; N = U*lam+dup; N += A'
                        DVE.tensor_tensor(Uj3[:, h0:h1, :],
                                          Pv[:, h0:h1, 1:cl + 1],
                                          Pv[:, h0 + 1:h1 + 1, 0:cl], ADD)
                        DVE.scalar_tensor_tensor(
                            U[j][:, h0 * cl:h1 * cl], U[j][:, h0 * cl:h1 * cl],
                            LAM, Pv[:, h0 + 1:h1 + 1, 1:cl + 1], MULT, ADD)
                        DVE.tensor_tensor(Nj3[:, h0:h1, :], Uj3[:, h0:h1, :],
                                          Aj3[:, h0:h1, :], ADD)

                # ---- j=1 + EC + u-add: row-half wavefront for the tail ----
                last = _step == t_steps - 1
                rl, cl = LD[1]
                r2, c2 = LD[2]
                Pv = _v3(P[1][:, :], cl + 2)
                N23 = _v3(N[2][:, 0:r2 * c2], c2)
                U13 = _v3(U[1][:, :], cl)
                A13 = _v3(A[1][:, :], cl)
                for rh in range(2):
                    h0, h1 = 8 * rh, 8 * rh + 8
                    # prolong P1 rows [h0+1, h1+1) <- dup(N2 rows [4rh,4rh+4))
                    # col-split: first ACT op covers the left half only
                    Vr = Pv[:, h0 + 1:h1 + 1, 1:cl + 1].rearrange(
                        "p (r2 a) c -> p r2 a c", a=2).rearrange(
                        "p r2 a (c2 b) -> p r2 a c2 b", b=2)
                    c2h = c2
                    for ch in range(1):
                        Edup = N23[:, 4 * rh:4 * rh + 4,
                                   c2h * ch:c2h * (ch + 1)].unsqueeze(3).\
                            broadcast_to([128, 4, c2h, 2])
                        Vc = Vr[:, :, :, c2h * ch:c2h * (ch + 1), :]
                        ACT.activation(Vc[:, :, 0, :, :], Edup, COPY_F)
                        ACT.activation(Vc[:, :, 1, :, :], Edup, COPY_F)
                    if rh == 0:
                        Elast = N23[:, r2 - 1, :].unsqueeze(2).broadcast_to(
                            [128, c2, 2])
                        PE.matmul(psum[:, 0:cl], zsh[:, 0:128], Elast)
                        DVE.tensor_copy(Pv[:, 0, 1:cl + 1], psum[:, 0:cl])
                    # DVE chain for this half
                    DVE.tensor_tensor(U13[:, h0:h1, :],
                                      Pv[:, h0:h1, 1:cl + 1],
                                      Pv[:, h0 + 1:h1 + 1, 0:cl], ADD)
                    DVE.scalar_tensor_tensor(
                        U[1][:, h0 * cl:h1 * cl], U[1][:, h0 * cl:h1 * cl],
                        LAM, Pv[:, h0 + 1:h1 + 1, 1:cl + 1], MULT, ADD)
                    DVE.tensor_tensor(N13[:, h0:h1, :], U13[:, h0:h1, :],
                                      A13[:, h0:h1, :], ADD)
                    # EC = coldup(N1 half) (ACT copy); u += rowdup(EC) (DVE)
                    E1dup = N13[:, h0:h1, :].unsqueeze(3).broadcast_to(
                        [128, 8, WCOL >> 1, 2])
                    ECr = EC3[:, h0:h1, :].rearrange(
                        "p r (cp two) -> p r cp two", two=2)
                    ACT.activation(ECr, E1dup, COPY_F)
                    if not last:
                        ECb = EC3[:, h0:h1, :].unsqueeze(2).broadcast_to(
                            [128, 8, 2, WCOL])
                        if rh == 0:
                            DVE.tensor_tensor(ur[:, h0:h1, :, :],
                                              ur[:, h0:h1, :, :], ECb, ADD)
                        else:
                            # ride the idle DMA engines: u += rowdup(EC)
                            # via SWDGE accumulate; DVE races into next step.
                            # Row-quarters so next-step t_rows waits less.
                            for qq in (h0, h0 + 4):
                                GPS.dma_start(ur[:, qq:qq + 4, 0, :],
                                              EC3[:, qq:qq + 4, :],
                                              accum_op=ADD)
                                GPS.dma_start(ur[:, qq:qq + 4, 1, :],
                                              EC3[:, qq:qq + 4, :],
                                              accum_op=ADD)
                    else:
                        # 4-row final adds so the store stream starts early
                        st_eng = (GPS, nc.sync, ACT, nc.sync)
                        for sub in range(4):
                            q0 = h0 + 2 * sub
                            ECb = EC3[:, q0:q0 + 2, :].unsqueeze(2).\
                                broadcast_to([128, 2, 2, WCOL])
                            DVE.tensor_tensor(ur[:, q0:q0 + 2, :, :],
                                              ur[:, q0:q0 + 2, :, :], ECb, ADD)
                            r0 = 16 * rh + 4 * sub
                            st_eng[sub].dma_start(
                                out_dram[:, r0:r0 + 4, :],
                                uav[:, 1 + r0:5 + r0, :])

    nc.finalize()
    return nc


def _consts_np(bs: float, br: float) -> np.ndarray:
    # bias for A'_j = -(alpha_j*S_j + gamma_j): store -gamma_j
    c = np.zeros(8, np.float32)
    c[0] = np.float32(-(bs + br) / DIAG)
    for j in range(2, 6):
        c[j - 1] = np.float32(-j * br / DIAG)
    return np.tile(c[None, :], (128, 1))


def kernel(u, b_smooth, b_res, t):
    t = int(np.asarray(t))
    u = np.ascontiguousarray(np.asarray(u, np.float32))
    bs = float(np.asarray(b_smooth).reshape(-1)[0])
    br = float(np.asarray(b_res).reshape(-1)[0])
    u2 = u[0, 0]

    if t not in _NC_CACHE:
        _NC_CACHE[t] = _build(t)
    nc = _NC_CACHE[t]

    consts = _consts_np(bs, br)
    u16 = u2.astype(np.float16)
    in_maps = []
    for k in range(8):
        w = u16[:, 0:WCOL] if k == 0 else u16[:, OWN * k - HALO: OWN * k + OWN]
        in_maps.append({"u": np.ascontiguousarray(w), "consts": consts})

    res = run_bass_kernel_spmd(nc, in_maps, list(range(8)))
    outs = []
    for k in range(8):
        o = res.results[k]["out"]
        outs.append(o[:, 0:OWN] if k == 0 else o[:, HALO:WCOL])
    full = np.concatenate(outs, axis=1)
    return full[None, None].astype(np.float32)


# revision 44
# speedup vs baseline: 1.0249x; 1.0061x over previous
"""Multigrid F-cycle advection smoother on 8 TRN2 NeuronCores.

Strategy (zero-communication, redundant compute):
  - Column-shard: core k computes a [4096, 544] window of u:
    core 0 -> cols [0, 544), core k>0 -> cols [512k-32, 512k+512).
    Host keeps cols [0:512) from core 0, [32:544) from others.
    (Halo-32 windowing error ~5e-3 rel, validated vs f32 numpy ref.)
  - Host converts u to f16; DRAM I/O is f16 (halves DMA bytes, lets
    loads/stores use SP/ACT HWDGE queues in addition to GPSIMD SWDGE).
  - On-chip layout: partition p holds rows [32p, 32p+32); free = [row, col].
  - All intermediates f16 (f32 for the two coarsest restriction levels).
  - The error chain is NEGATED (N_j = -E_j) so the final correction is a
    plain add of a plain dup:  A'_j = -alpha_j*S_j - gamma_j (ACT affine),
      N_j = dup(N_{j+1}) + lam*(up0+left0 of dup) + A'_j
          = TT(up0+left0) ; stt(*lam + dup) ; TT(+A'_j)
    EC = coldup(N_1) is a pure copy; u += rowdup(EC) is a 2x TT.
  - TS = SQ1*T on ACT (quarter chunks); u += TS runs on GPSIMD.
  - Up-chain j=4..2 and j=1 run as row-half wavefronts: ACT prolongs one
    half while DVE chews the other; PSUM halo evacuations on DVE.
  - On non-final steps the second u += rowdup(EC) half rides the idle DMA
    engines as a GPSIMD SWDGE accumulate, freeing DVE for the next step.
  - Cross-partition halo rows via TensorE identity matmul.
"""
import numpy as np

import concourse.bass as bass
import concourse.bacc as bacc
import concourse.mybir as mybir
from concourse import tile
from concourse.bass_utils import run_bass_kernel_spmd
from concourse.masks import make_identity

F32 = mybir.dt.float32
F16 = mybir.dt.float16
ADD = mybir.AluOpType.add
MULT = mybir.AluOpType.mult
COPY_F = mybir.ActivationFunctionType.Copy
IDENT_F = mybir.ActivationFunctionType.Identity

NROW, WCOL, OWN, HALO = 4096, 544, 512, 32
DIAG = 1.2
LAM = float(np.float32(0.1 / DIAG))   # lam = SQ1 = 0.1/DIAG
SQ1 = LAM
ALPHA = {j: float(np.float32(-0.1 * 4.0 ** (-j) / DIAG)) for j in range(1, 6)}
LD = {j: (32 >> j, WCOL >> j) for j in range(1, 6)}  # per-partition (rl, cl)

_NC_CACHE = {}


def _v3(ap, c):
    return ap.rearrange("p (r c) -> p r c", c=c)


def _build(t_steps: int):
    nc = bacc.Bacc(None, target_bir_lowering=False)
    u_ext = nc.declare_dram_parameter("u", [NROW, WCOL], F16, isOutput=False)
    c_ext = nc.declare_dram_parameter("consts", [128, 8], F32, isOutput=False)
    out_ext = nc.declare_dram_parameter("out", [NROW, WCOL], F16, isOutput=True)

    u_dram = u_ext[:, :].rearrange("(p r) c -> p r c", p=128)
    out_dram = out_ext[:, :].rearrange("(p r) c -> p r c", p=128)

    DVE, GPS, ACT, PE = nc.vector, nc.gpsimd, nc.scalar, nc.tensor

    with tile.TileContext(nc) as tc:
        with tc.tile_pool(name="sb", bufs=1) as pool, \
             tc.tile_pool(name="ps", bufs=1, space="PSUM") as ppool:
            ua = pool.tile([128, 33 * WCOL], F16, name="ua")   # row 0 = up halo
            T = pool.tile([128, 32 * WCOL], F16, name="t")
            TS = pool.tile([128, 32 * WCOL], F16, name="ts")
            RP = pool.tile([128, 16 * WCOL], F16, name="rp")   # also EC at step end
            S = {1: pool.tile([128, 16 * (WCOL >> 1)], F16, name="s1")}
            RA = {}
            for j in range(2, 6):
                rl, cl = LD[j]
                dt = F32 if j >= 4 else F16
                RA[j] = pool.tile([128, max(rl * 2 * cl, 8)], dt, name=f"ra{j}")
                S[j] = pool.tile([128, max(rl * cl, 8)], dt, name=f"s{j}")
            A = {j: pool.tile([128, max(LD[j][0] * LD[j][1], 8)], F16,
                              name=f"a{j}") for j in range(1, 5)}
            # N_j = -E_j tiles; scratch U_j for the shift sums
            N = {j: pool.tile([128, max(LD[j][0] * LD[j][1], 8)], F16, name=f"n{j}")
                 for j in range(1, 6)}
            U = {j: pool.tile([128, max(LD[j][0] * LD[j][1], 8)], F16, name=f"u{j}")
                 for j in range(1, 5)}
            P = {j: pool.tile([128, (1 + LD[j][0]) * (LD[j][1] + 2)], F16,
                              name=f"p{j}") for j in range(1, 5)}
            zsh = pool.tile([128, 130], F16, name="zsh")
            consts = pool.tile([128, 8], F32, name="k")
            psum = ppool.tile([128, 1024], F32, name="hps")

            nc.sync.dma_start(consts[:, :], c_ext[:, :])
            DVE.memset(zsh[:, :], 0.0)
            make_identity(nc, zsh[:, 1:129], nomemset=True)
            uav = _v3(ua[:, :], WCOL)
            # chunked load: 8 row-groups of 4 rows so first-step compute
            # starts ~2us after the first chunk lands
            ld_eng = [GPS, nc.sync, ACT, nc.sync]
            for ch in range(8):
                ld_eng[ch % 4].dma_start(uav[:, 1 + 4 * ch:5 + 4 * ch, :],
                                         u_dram[:, 4 * ch:4 + 4 * ch, :])

            g_ap = {j: consts[:, j - 1:j] for j in range(1, 6)}
            # zero left-pad column of every P tile (once; never rewritten)
            for j in range(1, 5):
                rl, cl = LD[j]
                Pv = _v3(P[j][:, :], cl + 2)
                zc = Pv[:, :, 0:1].rearrange("p r one -> p (r one)")
                DVE.memset(zc, 0.0)

            T3 = _v3(T[:, :], WCOL)
            TS3 = _v3(TS[:, :], WCOL)
            Tp = T[:, :].rearrange("p (rp two c) -> p rp two c", two=2, c=WCOL)
            RP3 = _v3(RP[:, :], WCOL)
            RPc = RP[:, :].rearrange("p (r cp two) -> p r cp two", r=16, two=2)
            S13 = _v3(S[1][:, :], WCOL >> 1)
            ur = uav[:, 1:33, :].rearrange("p (rp two) c -> p rp two c", two=2)
            N13 = _v3(N[1][:, :], WCOL >> 1)
            EC3 = RP3  # EC aliases RP ([p, 16, WCOL])

            def t_rows(r0, r1):
                """T = u_up + u_left for stripe rows [r0, r1)."""
                DVE.tensor_tensor(T3[:, r0:r1, 1:WCOL], uav[:, r0:r1, 1:WCOL],
                                  uav[:, r0 + 1:r1 + 1, 0:WCOL - 1], ADD)
                DVE.tensor_tensor(T3[:, r0:r1, 0:1], uav[:, r0:r1, 0:1],
                                  uav[:, r0 + 1:r1 + 1, 0:1], ADD)

            def u_halo():
                PE.matmul(psum[:, 0:512], zsh[:, 0:128], uav[:, 32, 0:512])
                PE.matmul(psum[:, 512:WCOL], zsh[:, 0:128], uav[:, 32, 512:WCOL])
                ACT.activation(uav[:, 0, :], psum[:, 0:WCOL], COPY_F)

            def p0_copy():
                # global-top replicate BC for partition 0 (DVE: tiny, and
                # keeps it out of ACT's TS-busy queue)
                DVE.tensor_copy(uav[0:1, 0, :], uav[0:1, 1, :])

            for _step in range(t_steps):
                first = _step == 0
                if first:
                    # overlap the load: rows 1.. of each chunk need no halo
                    t_rows(1, 4)
                    for ch in range(1, 8):
                        t_rows(4 * ch, 4 * ch + 4)
                    u_halo()           # waits only on last-loaded chunk
                    p0_copy()
                    t_rows(0, 1)
                else:
                    u_halo()
                    t_rows(1, 16)
                    t_rows(16, 24)
                    t_rows(24, 32)
                    p0_copy()
                    t_rows(0, 1)

                # ---- TS = SQ1 * T (ACT); u += TS (GPS) ----
                # rows 0:28 now; rows 28:32 deferred into ACT's idle window
                # during the j=1 prolongation (keeps ACT free when S5 lands)
                def ts_g(r0, r1):
                    ACT.activation(TS3[:, r0:r1, :], T3[:, r0:r1, :], COPY_F,
                                   scale=SQ1)
                    u_flat = ua[:, (1 + r0) * WCOL:(1 + r1) * WCOL]
                    GPS.tensor_tensor(u_flat, u_flat,
                                      TS[:, r0 * WCOL:r1 * WCOL], ADD)

                for q in range(4):
                    ts_g(8 * q, 8 * q + 8)

                # ---- restriction: RP = rowpair(T); S1 = colpair(RP) ----
                DVE.tensor_tensor(RP3[:, :, :], Tp[:, :, 0, :], Tp[:, :, 1, :],
                                  ADD)
                DVE.tensor_tensor(S13[:, :, :], RPc[:, :, :, 0],
                                  RPc[:, :, :, 1], ADD)
                prev = S[1]
                for j in range(2, 6):
                    rl, cl = LD[j]
                    pv = prev[:, 0:rl * 4 * cl].rearrange(
                        "p (rp two c) -> p rp two c", two=2, c=2 * cl)
                    ra3 = _v3(RA[j][:, 0:rl * 2 * cl], 2 * cl)
                    DVE.tensor_tensor(ra3[:, :, :], pv[:, :, 0, :],
                                      pv[:, :, 1, :], ADD)
                    rac = RA[j][:, 0:rl * 2 * cl].rearrange(
                        "p (r cp two) -> p r cp two", r=rl, two=2)
                    s3 = _v3(S[j][:, 0:rl * cl], cl)
                    DVE.tensor_tensor(s3[:, :, :], rac[:, :, :, 0],
                                      rac[:, :, :, 1], ADD)
                    prev = S[j]

                # ---- A'_j = -(alpha_j*S_j + gamma_j) (ACT affine) ----
                # N5 = A'_5 directly; emitted in need order (coarse first)
                # N5 on DVE: runs back-to-back after S5 in the DVE queue,
                # skipping a cross-engine hop on the critical path
                n5 = LD[5][1]
                DVE.tensor_scalar(N[5][:, 0:n5], S[5][:, 0:n5], -ALPHA[5],
                                  g_ap[5], MULT, ADD)
                for j in range(4, 0, -1):
                    rl, cl = LD[j]
                    ACT.activation(A[j][:, 0:rl * cl], S[j][:, 0:rl * cl],
                                   IDENT_F, bias=g_ap[j], scale=-ALPHA[j])

                # ---- up chain j=4..2: N_j = dup + lam*(up0+left0) + A' ----
                # row-half wavefront: ACT prolongs half rh while DVE chews
                # the previous half, so ACT latency hides behind DVE work.
                for j in range(4, 1, -1):
                    rl, cl = LD[j]
                    r2, c2 = LD[j + 1]
                    rlh, r2h = rl // 2, max(r2 // 2, 1)
                    Pv = _v3(P[j][:, :], cl + 2)
                    Nin3 = _v3(N[j + 1][:, 0:r2 * c2], c2)
                    Uj3 = _v3(U[j][:, 0:rl * cl], cl)
                    Nj3 = _v3(N[j][:, 0:rl * cl], cl)
                    Aj3 = _v3(A[j][:, 0:rl * cl], cl)
                    for rh in range(2):
                        h0, h1 = rlh * rh, rlh * (rh + 1)
                        # prolong P rows [h0+1, h1+1) <- dup(N_{j+1} rows)
                        if rlh == 1:
                            # j=4: each half is one row, dup of N5 row 0
                            Ed1 = Nin3[:, 0, :].unsqueeze(2).broadcast_to(
                                [128, c2, 2])
                            Vr1 = Pv[:, h0 + 1, 1:cl + 1].rearrange(
                                "p (c2 b) -> p c2 b", b=2)
                            ACT.activation(Vr1, Ed1, COPY_F)
                        else:
                            Vr = Pv[:, h0 + 1:h1 + 1, 1:cl + 1].rearrange(
                                "p (rr a) c -> p rr a c", a=2).rearrange(
                                "p rr a (c2 b) -> p rr a c2 b", b=2)
                            Edup = Nin3[:, r2h * rh:r2h * (rh + 1), :].\
                                unsqueeze(3).broadcast_to([128, r2h, c2, 2])
                            ACT.activation(Vr[:, :, 0, :, :], Edup, COPY_F)
                            ACT.activation(Vr[:, :, 1, :, :], Edup, COPY_F)
                        if rh == 0:
                            # halo row via PE; evacuation on DVE
                            Elast = Nin3[:, r2 - 1, :].unsqueeze(2).\
                                broadcast_to([128, c2, 2])
                            PE.matmul(psum[:, 0:cl], zsh[:, 0:128], Elast)
                            DVE.tensor_copy(Pv[:, 0, 1:cl + 1], psum[:, 0:cl])
                        # chain (DVE): U = up0+left0; N = U*lam+dup; N += A'
                        DVE.tensor_tensor(Uj3[:, h0:h1, :],
                                          Pv[:, h0:h1, 1:cl + 1],
                                          Pv[:, h0 + 1:h1 + 1, 0:cl], ADD)
                        DVE.scalar_tensor_tensor(
                            U[j][:, h0 * cl:h1 * cl], U[j][:, h0 * cl:h1 * cl],
                            LAM, Pv[:, h0 + 1:h1 + 1, 1:cl + 1], MULT, ADD)
                        DVE.tensor_tensor(Nj3[:, h0:h1, :], Uj3[:, h0:h1, :],
                                          Aj3[:, h0:h1, :], ADD)

                # ---- j=1 + EC + u-add: row-half wavefront for the tail ----
                last = _step == t_steps - 1
                rl, cl = LD[1]
                r2, c2 = LD[2]
                Pv = _v3(P[1][:, :], cl + 2)
                N23 = _v3(N[2][:, 0:r2 * c2], c2)
                U13 = _v3(U[1][:, :], cl)
                A13 = _v3(A[1][:, :], cl)
                for rh in range(2):
                    h0, h1 = 8 * rh, 8 * rh + 8
                    # prolong P1 rows [h0+1, h1+1) <- dup(N2 rows [4rh,4rh+4))
                    # col-split: first ACT op covers the left half only
                    Vr = Pv[:, h0 + 1:h1 + 1, 1:cl + 1].rearrange(
                        "p (r2 a) c -> p r2 a c", a=2).rearrange(
                        "p r2 a (c2 b) -> p r2 a c2 b", b=2)
                    c2h = c2
                    for ch in range(1):
                        Edup = N23[:, 4 * rh:4 * rh + 4,
                                   c2h * ch:c2h * (ch + 1)].unsqueeze(3).\
                            broadcast_to([128, 4, c2h, 2])
                        Vc = Vr[:, :, :, c2h * ch:c2h * (ch + 1), :]
                        ACT.activation(Vc[:, :, 0, :, :], Edup, COPY_F)
                        ACT.activation(Vc[:, :, 1, :, :], Edup, COPY_F)
                    if rh == 0:
                        Elast = N23[:, r2 - 1, :].unsqueeze(2).broadcast_to(
                            [128, c2, 2])
                        PE.matmul(psum[:, 0:cl], zsh[:, 0:128], Elast)
                        DVE.tensor_copy(Pv[:, 0, 1:cl + 1], psum[:, 0:cl])
                    # DVE chain for this half
                    DVE.tensor_tensor(U13[:, h0:h1, :],
                                      Pv[:, h0:h1, 1:cl + 1],
                                      Pv[:, h0 + 1:h1 + 1, 0:cl], ADD)
                    DVE.scalar_tensor_tensor(
                        U[1][:, h0 * cl:h1 * cl], U[1][:, h0 * cl:h1 * cl],
                        LAM, Pv[:, h0 + 1:h1 + 1, 1:cl + 1], MULT, ADD)
                    DVE.tensor_tensor(N13[:, h0:h1, :], U13[:, h0:h1, :],
                                      A13[:, h0:h1, :], ADD)
                    # EC = coldup(N1 half) (ACT copy); u += rowdup(EC) (DVE)
                    E1dup = N13[:, h0:h1, :].unsqueeze(3).broadcast_to(
                        [128, 8, WCOL >> 1, 2])
                    ECr = EC3[:, h0:h1, :].rearrange(
                        "p r (cp two) -> p r cp two", two=2)
                    ACT.activation(ECr, E1dup, COPY_F)
                    if not last:
                        ECb = EC3[:, h0:h1, :].unsqueeze(2).broadcast_to(
                            [128, 8, 2, WCOL])
                        if rh == 0:
                            DVE.tensor_tensor(ur[:, h0:h1, :, :],
                                              ur[:, h0:h1, :, :], ECb, ADD)
                        else:
                            # ride the idle DMA engines: u += rowdup(EC)
                            # via SWDGE accumulate; DVE races into next step.
                            # Row-quarters so next-step t_rows waits less.
                            for qq in (h0, h0 + 4):
                                GPS.dma_start(ur[:, qq:qq + 4, 0, :],
                                              EC3[:, qq:qq + 4, :],
                                              accum_op=ADD)
                                GPS.dma_start(ur[:, qq:qq + 4, 1, :],
                                              EC3[:, qq:qq + 4, :],
                                              accum_op=ADD)
                    else:
                        # 4-row final adds so the store stream starts early
                        st_eng = (GPS, nc.sync, ACT, nc.sync)
                        for sub in range(4):
                            q0 = h0 + 2 * sub
                            ECb = EC3[:, q0:q0 + 2, :].unsqueeze(2).\
                                broadcast_to([128, 2, 2, WCOL])
                            DVE.tensor_tensor(ur[:, q0:q0 + 2, :, :],
                                              ur[:, q0:q0 + 2, :, :], ECb, ADD)
                            r0 = 16 * rh + 4 * sub
                            st_eng[sub].dma_start(
                                out_dram[:, r0:r0 + 4, :],
                                uav[:, 1 + r0:5 + r0, :])

    nc.finalize()
    return nc


def _consts_np(bs: float, br: float) -> np.ndarray:
    # bias for A'_j = -(alpha_j*S_j + gamma_j): store -gamma_j
    c = np.zeros(8, np.float32)
    c[0] = np.float32(-(bs + br) / DIAG)
    for j in range(2, 6):
        c[j - 1] = np.float32(-j * br / DIAG)
    return np.tile(c[None, :], (128, 1))


def kernel(u, b_smooth, b_res, t):
    t = int(np.asarray(t))
    u = np.ascontiguousarray(np.asarray(u, np.float32))
    bs = float(np.asarray(b_smooth).reshape(-1)[0])
    br = float(np.asarray(b_res).reshape(-1)[0])
    u2 = u[0, 0]

    if t not in _NC_CACHE:
        _NC_CACHE[t] = _build(t)
    nc = _NC_CACHE[t]

    consts = _consts_np(bs, br)
    u16 = u2.astype(np.float16)
    in_maps = []
    for k in range(8):
        w = u16[:, 0:WCOL] if k == 0 else u16[:, OWN * k - HALO: OWN * k + OWN]
        in_maps.append({"u": np.ascontiguousarray(w), "consts": consts})

    res = run_bass_kernel_spmd(nc, in_maps, list(range(8)))
    outs = []
    for k in range(8):
        o = res.results[k]["out"]
        outs.append(o[:, 0:OWN] if k == 0 else o[:, HALO:WCOL])
    full = np.concatenate(outs, axis=1)
    return full[None, None].astype(np.float32)
